# revision 1
# baseline (speedup 1.0000x reference)
"""Trainium2 Bass kernel for the per-pixel locally-connected MLP (dense_mlp).

Reference computation (per batch b, pixel (h,w)):
    x0 = coor (2-vector, shared by all pixels)
    h1 = relu(W0 @ x0)        W0 = weight[b, 0:32].reshape(16, 2)   per pixel
    h2 = relu(W1 @ h1)        W1 = weight[b, 32:288].reshape(16,16) per pixel
    y  = W2 @ h2 + bias       W2 = weight[b, 288:336].reshape(3,16), bias = weight[b,336]
Output: [4, 3, 256, 256] float32.

Sharding: 8 cores, core k handles batch k//2, image rows (k%2)*128:(k%2+1)*128
=> per-core weight shard [337, 32768] (channels x pixels); no cross-core comm.

Implementation notes:
- Channels live on SBUF partitions, pixels on the free axis, so every weight
  load is a wide contiguous DMA; weights are cast to fp16 on the host to halve
  HBM traffic (the kernel is purely memory-bound; rel err ~5e-4).
- The per-pixel matvecs are elementwise multiplies (VectorE) plus
  partition-axis reductions (TensorE matmuls against small host-built 0/1
  selection matrices; `coor` is folded into the first matmul's stationary
  matrix, the bias channel into the last one's moving operand). Matmul
  operands use float32r (TF32-like) for 4x PE throughput over fp32.
- Sub-chunks of 512 px (one PSUM bank) are processed in pairs that share PSUM
  banks: the pair's two halves land at partitions 0:16 / 32:48 of one bank via
  zero-padded stationaries accumulating at a base-0 dst, so each ScalarE
  relu / output-copy covers two chunks in one op.
- Work is emitted stage-major over 4096-px macro-tiles (3 merged HWDGE loads
  per macro + SWDGE bias/output DMAs on gpsimd) so the in-order engine queues
  pipeline across sub-chunks; pool buffer counts are sized to keep 2-3 macros
  in flight without deadlocking the Tile scheduler.
"""

import sys

for _p in ("/opt/trn_rl_repo", "/root/.axon_site/_ro/trn_rl_repo"):
    if _p not in sys.path:
        sys.path.append(_p)

import numpy as np

import concourse.bass as bass
import concourse.tile as tile
from concourse import bacc, mybir
from concourse.bass_utils import run_bass_kernel_spmd

# ---------------------------------------------------------------- constants
B, H, W = 4, 256, 256
N_CH = 337            # 32 (L0) + 256 (L1) + 48 (L2) + 1 (bias)
N_CORES = 8
PIX = (B * H * W) // N_CORES  # 32768 pixels per core
F = 512               # pixels per compute chunk (one PSUM bank of fp32)
N_CHUNKS = PIX // F

FP32 = mybir.dt.float32
FP32R = mybir.dt.float32r
FP16 = mybir.dt.float16


def _const_mats(coor: np.ndarray) -> dict[str, np.ndarray]:
    """Small stationary matrices for the TensorE reductions."""
    cx, cy = float(coor[0]), float(coor[1])
    # S0 for a pair-stacked moving operand t0 [64, F]: rows 0:32 are the
    # even chunk's L0 weights -> cols 0:16, rows 32:64 the odd chunk's ->
    # cols 32:48. One matmul produces both halves of h1pre.
    s0 = np.zeros((64, 48), np.float32)
    for h in range(2):
        for i in range(16):
            s0[32 * h + 2 * i, 32 * h + i] = cx
            s0[32 * h + 2 * i + 1, 32 * h + i] = cy
    r8 = np.zeros((16, 128), np.float32)      # h1rep[m] = h1[m % 16]
    for m in range(128):
        r8[m % 16, m] = 1.0
    m1a = np.zeros((2, 128, 48), np.float32)  # h2pre[j] += sum_i prodA[16j+i]
    m1b = np.zeros((2, 128, 48), np.float32)
    for h in range(2):
        for k in range(128):
            m1a[h, k, 32 * h + k // 16] = 1.0
            m1b[h, k, 32 * h + 8 + k // 16] = 1.0
    # pair-fused layer-2: moving operand is h2pair [48,F] with chunk A's h2
    # at rows 0:16 and chunk B's at rows 32:48 (rows 16:32 are junk)
    r3_2 = np.zeros((48, 96), np.float32)     # h2rep2[k] = h2(k//48)[k % 16]
    for k in range(96):
        r3_2[(0 if k < 48 else 32) + k % 16, k] = 1.0
    m2b2 = np.zeros((98, 6), np.float32)      # y[h*3+j] = sum prodC + bias
    for k in range(96):
        m2b2[k, (k // 48) * 3 + (k % 48) // 16] = 1.0
    m2b2[96, 0:3] = 1.0                       # bias row, even chunk
    m2b2[97, 3:6] = 1.0                       # bias row, odd chunk
    return {"s0": s0.astype(np.float16), "r8": r8, "m1a": m1a, "m1b": m1b,
            "r3_2": r3_2, "m2b2": m2b2}


def build_nc(repeat: int = 1):
    """Build the per-core Bass program. `repeat` re-runs the whole kernel
    body sequentially (used only for differential HW timing)."""
    nc = bacc.Bacc(None, target_bir_lowering=False)

    w = nc.declare_dram_parameter("w", [N_CH, PIX], FP16, isOutput=False)
    wb = nc.declare_dram_parameter("wb", [1, PIX], FP32R, isOutput=False)
    out = nc.declare_dram_parameter("out", [3, PIX], FP32, isOutput=True)
    c_s0 = nc.declare_dram_parameter("s0", [64, 48], FP16, isOutput=False)
    c_r8 = nc.declare_dram_parameter("r8", [16, 128], FP32R, isOutput=False)
    c_m1a = nc.declare_dram_parameter("m1a", [2, 128, 48], FP32R, isOutput=False)
    c_m1b = nc.declare_dram_parameter("m1b", [2, 128, 48], FP32R, isOutput=False)
    c_r3_2 = nc.declare_dram_parameter("r3_2", [48, 96], FP32R, isOutput=False)
    c_m2b2 = nc.declare_dram_parameter("m2b2", [98, 6], FP32R, isOutput=False)

    G = 8                      # chunks per software-pipeline group
    with tile.TileContext(nc) as tc:
        with (
            tc.tile_pool(name="consts", bufs=1) as consts,
            tc.tile_pool(name="loads", bufs=2) as loads,
            tc.tile_pool(name="acts", bufs=4) as acts,
            tc.tile_pool(name="prods", bufs=3) as prods,
            tc.tile_pool(name="outs", bufs=2) as outs,
            tc.tile_pool(name="ps_sm16", bufs=2, space="PSUM") as ps_sm16,
            tc.tile_pool(name="ps_h2p", bufs=2, space="PSUM") as ps_h2p,
            tc.tile_pool(name="ps_rep", bufs=2, space="PSUM") as ps_rep
            ,tc.tile_pool(name="ps_y", bufs=2, space="PSUM") as ps_y,
        ):
            s0 = consts.tile([64, 48], FP16)
            r8 = consts.tile([48, 128], FP32R)   # rows 0:16 and 32:48 both
                                                 # hold R8 (for base 0/32)
            m1a = consts.tile([128, 2, 48], FP32R)
            m1b = consts.tile([128, 2, 48], FP32R)
            r3_2 = consts.tile([48, 96], FP32R)
            m2b2 = consts.tile([98, 6], FP32R)
            for t, d in ((s0, c_s0),
                         (r8[0:16, :], c_r8), (r8[32:48, :], c_r8),
                         (m1a, c_m1a.rearrange("h k m -> k h m")),
                         (m1b, c_m1b.rearrange("h k m -> k h m")),
                         (r3_2, c_r3_2), (m2b2, c_m2b2)):
                nc.sync.dma_start(out=t[:], in_=d[:])

            relu = mybir.ActivationFunctionType.Relu

            def body():
                # Macro-tile of G*F pixels; inside, sub-chunks are processed
                # in PAIRS sharing PSUM banks at partition offsets 0/32 (both
                # legal matmul base partitions). This halves ACT op count and
                # fuses all of layer 2 (rep, products, reduce+bias) per pair.
                FM = G * F
                NP = G // 2     # pairs per macro
                for g in range(N_CHUNKS // G):
                    mp = slice(g * FM, (g + 1) * FM)
                    sls = [slice(i * F, (i + 1) * F) for i in range(G)]
                    psl = [slice(p * F, (p + 1) * F) for p in range(NP)]

                    t0m = loads.tile([64, NP, F], FP16, tag="t0", name="t0m",
                                     bufs=3)
                    t1m = loads.tile([128, 2, FM], FP16, tag="t1", name="t1m", bufs=3)
                    # t2 pair-stacked: partition h*48+ch, free (pair, x);
                    # h = parity of the sub-chunk within its pair
                    t2m = loads.tile([96, NP, F], FP16, tag="t2", name="t2m", bufs=3)
                    pcm = prods.tile([98, NP, F], FP32R, tag="pcm", name="pcm",
                                     bufs=3)
                    for h in range(2):
                        nc.sync.dma_start(
                            out=t0m[32 * h:32 * h + 32, :, :],
                            in_=bass.AP(tensor=w[:].tensor,
                                        offset=g * FM + h * F,
                                        ap=[[PIX, 32], [2 * F, NP], [1, F]]))
                    nc.sync.dma_start(
                        out=t1m[:],
                        in_=w[32:288, mp].rearrange("(b p) x -> p b x", b=2))
                    for h in range(2):
                        nc.sync.dma_start(
                            out=t2m[48 * h:48 * h + 48, :, :],
                            in_=bass.AP(tensor=w[:].tensor,
                                        offset=288 * PIX + g * FM + h * F,
                                        ap=[[PIX, 48], [2 * F, NP], [1, F]]))
                        nc.sync.dma_start(
                            out=pcm[96 + h:97 + h, :, :],
                            in_=bass.AP(tensor=wb[:].tensor,
                                        offset=g * FM + h * F,
                                        ap=[[2 * F, NP], [1, F]]))

                    h1pre = {}
                    for p in range(NP):
                        h1pre[p] = ps_sm16.tile([48, F], FP32, tag="sm16",
                                                name="h1pre")
                        nc.tensor.matmul(h1pre[p][:], s0[:],
                                         t0m[:, p, :],
                                         start=True, stop=True)
                    h1 = {}
                    for p in range(NP):
                        h1[p] = acts.tile([48, F], FP32R, tag="h1", name="h1")
                        nc.scalar.activation(h1[p][:], h1pre[p][:], relu)
                    h1rep = {}
                    for p in range(NP):
                        for h in range(2):
                            h1rep[p, h] = ps_rep.tile([128, F], FP32,
                                                      tag="rep", name="h1rep")
                            nc.tensor.matmul(
                                h1rep[p, h][:],
                                r8[32 * h:32 * h + 16, :],
                                h1[p][32 * h:32 * h + 16, :],
                                start=True, stop=True)
                    h1r_sb = {}
                    for p in range(NP):
                        h1r_sb[p] = acts.tile([128, F], FP16, tag="h1r",
                                              name="h1r", bufs=3)
                        nc.scalar.copy(h1r_sb[p][:], h1rep[p, 1][:])
                    prodAB = {}
                    for p in range(NP):
                        for h in range(2):
                            prodAB[p, h] = prods.tile([128, 2, F], FP32R,
                                                      tag="prodAB",
                                                      name="prodAB", bufs=5)
                            rp = h1rep[p, h]
                            rep2 = bass.AP(tensor=rp.tensor,
                                           offset=rp[:].offset,
                                           ap=[rp[:].ap[0], [0, 2],
                                               rp[:].ap[1]])
                            if h == 0:
                                nc.vector.tensor_mul(
                                    prodAB[p, h][:],
                                    t1m[:, :, sls[2 * p + h]], rep2)
                            else:
                                sb = h1r_sb[p]
                                rep2s = bass.AP(tensor=sb.tensor,
                                                offset=sb[:].offset,
                                                ap=[sb[:].ap[0], [0, 2],
                                                    sb[:].ap[1]])
                                nc.gpsimd.tensor_mul(
                                    prodAB[p, h][:],
                                    t1m[:, :, sls[2 * p + h]], rep2s)
                    h2pre = {}
                    for p in range(NP):
                        h2pre[p] = ps_h2p.tile([48, F], FP32, tag="h2p",
                                               name="h2pre")
                        for h in range(2):
                            nc.tensor.matmul(h2pre[p][:],
                                             m1a[:, h, :], prodAB[p, h][:, 0, :],
                                             start=(h == 0), stop=False)
                            nc.tensor.matmul(h2pre[p][:],
                                             m1b[:, h, :], prodAB[p, h][:, 1, :],
                                             start=False, stop=(h == 1))
                    h2 = {}
                    for p in range(NP):
                        h2[p] = acts.tile([48, F], FP32R, tag="h2", name="h2")
                        nc.scalar.activation(h2[p][:], h2pre[p][:], relu)
                    h2rep = {}
                    for p in range(NP):
                        h2rep[p] = ps_rep.tile([96, F], FP32, tag="rep",
                                               name="h2rep")
                        nc.tensor.matmul(h2rep[p][:], r3_2[:], h2[p][:],
                                         start=True, stop=True)
                    for p in range(NP):
                        nc.vector.tensor_mul(pcm[0:96, p, :],
                                             t2m[:, p, :], h2rep[p][:])
                    y = {}
                    for p in range(NP):
                        y[p] = ps_y.tile([6, F], FP32, tag="y", name="y")
                        nc.tensor.matmul(y[p][:], m2b2[:], pcm[:, p, :],
                                         start=True, stop=True)
                    y_sb = outs.tile([6, NP, F], FP32, tag="ysb", name="ysb",
                                     bufs=2)
                    for p in range(NP):
                        nc.scalar.copy(y_sb[:, p, :], y[p][:])
                    # y_sb partition h*3+j, free (pair, x) ->
                    # out[j, g*FM + pair*2F + h*F + x]
                    for h in range(2):
                        nc.sync.dma_start(
                            out=bass.AP(tensor=out[:].tensor,
                                        offset=g * FM + h * F,
                                        ap=[[PIX, 3], [2 * F, NP], [1, F]]),
                            in_=y_sb[3 * h:3 * h + 3, :, :])

            if repeat == 1:
                body()
            else:
                with tc.For_i(0, repeat, 1):
                    body()

    nc.compile()
    return nc


_NC_CACHE: dict[int, object] = {}


def _get_nc(repeat: int = 1):
    if repeat not in _NC_CACHE:
        _NC_CACHE[repeat] = build_nc(repeat)
    return _NC_CACHE[repeat]


def make_in_maps(weight: np.ndarray, coor: np.ndarray) -> list[dict]:
    mats = _const_mats(coor)
    in_maps = []
    for k in range(N_CORES):
        b, hh = k // 2, k % 2
        shard = np.ascontiguousarray(
            weight[b, :, hh * 128:(hh + 1) * 128, :].reshape(N_CH, PIX),
            dtype=np.float16)
        in_maps.append({"w": shard,
                        "wb": weight[b, 336, hh * 128:(hh + 1) * 128, :]
                        .reshape(1, PIX).astype(np.float32), **mats})
    return in_maps


def assemble_out(results: list[dict]) -> np.ndarray:
    out = np.empty((B, 3, H, W), np.float32)
    for k in range(N_CORES):
        b, hh = k // 2, k % 2
        out[b, :, hh * 128:(hh + 1) * 128, :] = results[k]["out"].reshape(3, 128, W)
    return out


def kernel(input: np.ndarray, weight: np.ndarray, coor: np.ndarray) -> np.ndarray:
    nc = _get_nc(1)
    in_maps = make_in_maps(np.asarray(weight), np.asarray(coor))
    res = run_bass_kernel_spmd(nc, in_maps, core_ids=list(range(N_CORES)))
    return assemble_out(res.results)



# revision 8
# speedup vs baseline: 1.0019x; 1.0019x over previous
"""Trainium2 Bass kernel for the per-pixel locally-connected MLP (dense_mlp).

Reference computation (per batch b, pixel (h,w)):
    x0 = coor (2-vector, shared by all pixels)
    h1 = relu(W0 @ x0)        W0 = weight[b, 0:32].reshape(16, 2)   per pixel
    h2 = relu(W1 @ h1)        W1 = weight[b, 32:288].reshape(16,16) per pixel
    y  = W2 @ h2 + bias       W2 = weight[b, 288:336].reshape(3,16), bias = weight[b,336]
Output: [4, 3, 256, 256] float32.

Sharding: 8 cores, core k handles batch k//2, image rows (k%2)*128:(k%2+1)*128
=> per-core weight shard [337, 32768] (channels x pixels); no cross-core comm.

Implementation notes:
- Channels live on SBUF partitions, pixels on the free axis, so every weight
  load is a wide contiguous DMA; weights are cast to fp16 on the host to halve
  HBM traffic (the kernel is purely memory-bound; rel err ~5e-4).
- The per-pixel matvecs are elementwise multiplies (VectorE) plus
  partition-axis reductions (TensorE matmuls against small host-built 0/1
  selection matrices; `coor` is folded into the first matmul's stationary
  matrix, the bias channel into the last one's moving operand). Matmul
  operands use float32r (TF32-like) for 4x PE throughput over fp32.
- Sub-chunks of 512 px (one PSUM bank) are processed in pairs that share PSUM
  banks: the pair's two halves land at partitions 0:16 / 32:48 of one bank via
  zero-padded stationaries accumulating at a base-0 dst, so each ScalarE
  relu / output-copy covers two chunks in one op.
- Work is emitted stage-major over 4096-px macro-tiles (3 merged HWDGE loads
  per macro + SWDGE bias/output DMAs on gpsimd) so the in-order engine queues
  pipeline across sub-chunks; pool buffer counts are sized to keep 2-3 macros
  in flight without deadlocking the Tile scheduler.
"""

import sys

for _p in ("/opt/trn_rl_repo", "/root/.axon_site/_ro/trn_rl_repo"):
    if _p not in sys.path:
        sys.path.append(_p)

import numpy as np

import concourse.bass as bass
import concourse.tile as tile
from concourse import bacc, mybir
from concourse.bass_utils import run_bass_kernel_spmd

# ---------------------------------------------------------------- constants
B, H, W = 4, 256, 256
N_CH = 337            # 32 (L0) + 256 (L1) + 48 (L2) + 1 (bias)
N_CORES = 8
PIX = (B * H * W) // N_CORES  # 32768 pixels per core
F = 512               # pixels per compute chunk (one PSUM bank of fp32)
N_CHUNKS = PIX // F

FP32 = mybir.dt.float32
FP32R = mybir.dt.float32r
FP16 = mybir.dt.float16


def _const_mats(coor: np.ndarray) -> dict[str, np.ndarray]:
    """Small stationary matrices for the TensorE reductions."""
    cx, cy = float(coor[0]), float(coor[1])
    # S0 for a pair-stacked moving operand t0 [64, F]: rows 0:32 are the
    # even chunk's L0 weights -> cols 0:16, rows 32:64 the odd chunk's ->
    # cols 32:48. One matmul produces both halves of h1pre.
    s0 = np.zeros((64, 48), np.float32)
    for h in range(2):
        for i in range(16):
            s0[32 * h + 2 * i, 32 * h + i] = cx
            s0[32 * h + 2 * i + 1, 32 * h + i] = cy
    r8 = np.zeros((16, 128), np.float32)      # h1rep[m] = h1[m % 16]
    for m in range(128):
        r8[m % 16, m] = 1.0
    m1a = np.zeros((2, 128, 48), np.float32)  # h2pre[j] += sum_i prodA[16j+i]
    m1b = np.zeros((2, 128, 48), np.float32)
    for h in range(2):
        for k in range(128):
            m1a[h, k, 32 * h + k // 16] = 1.0
            m1b[h, k, 32 * h + 8 + k // 16] = 1.0
    # pair-fused layer-2: moving operand is h2pair [48,F] with chunk A's h2
    # at rows 0:16 and chunk B's at rows 32:48 (rows 16:32 are junk)
    r3_2 = np.zeros((48, 96), np.float32)     # h2rep2[k] = h2(k//48)[k % 16]
    for k in range(96):
        r3_2[(0 if k < 48 else 32) + k % 16, k] = 1.0
    m2b2 = np.zeros((98, 6), np.float32)      # y[h*3+j] = sum prodC + bias
    for k in range(96):
        m2b2[k, (k // 48) * 3 + (k % 48) // 16] = 1.0
    m2b2[96, 0:3] = 1.0                       # bias row, even chunk
    m2b2[97, 3:6] = 1.0                       # bias row, odd chunk
    return {"s0": s0.astype(np.float16), "r8": r8, "m1a": m1a, "m1b": m1b,
            "r3_2": r3_2, "m2b2": m2b2}


def build_nc(repeat: int = 1, mode: str = "full"):
    """Build the per-core Bass program. `repeat` re-runs the whole kernel
    body sequentially (used only for differential HW timing). `mode` limits
    which instruction classes are emitted (timing experiments only):
    full | dma (loads/stores only) | compute (no HBM traffic) | mm
    (matmuls only)."""
    do_dma = mode in ("full", "dma")
    do_vec = mode in ("full", "compute")
    do_mm = mode in ("full", "compute", "mm")
    nc = bacc.Bacc(None, target_bir_lowering=False)

    w = nc.declare_dram_parameter("w", [N_CH, PIX], FP16, isOutput=False)
    wb = nc.declare_dram_parameter("wb", [1, PIX], FP32R, isOutput=False)
    out = nc.declare_dram_parameter("out", [3, PIX], FP32, isOutput=True)
    c_s0 = nc.declare_dram_parameter("s0", [64, 48], FP16, isOutput=False)
    c_r8 = nc.declare_dram_parameter("r8", [16, 128], FP32R, isOutput=False)
    c_m1a = nc.declare_dram_parameter("m1a", [2, 128, 48], FP32R, isOutput=False)
    c_m1b = nc.declare_dram_parameter("m1b", [2, 128, 48], FP32R, isOutput=False)
    c_r3_2 = nc.declare_dram_parameter("r3_2", [48, 96], FP32R, isOutput=False)
    c_m2b2 = nc.declare_dram_parameter("m2b2", [98, 6], FP32R, isOutput=False)

    G = 8                      # chunks per software-pipeline group
    with tile.TileContext(nc) as tc:
        with (
            tc.tile_pool(name="consts", bufs=1) as consts,
            tc.tile_pool(name="loads", bufs=2) as loads,
            tc.tile_pool(name="acts", bufs=4) as acts,
            tc.tile_pool(name="prods", bufs=3) as prods,
            tc.tile_pool(name="outs", bufs=2) as outs,
            tc.tile_pool(name="ps_sm16", bufs=2, space="PSUM") as ps_sm16,
            tc.tile_pool(name="ps_h2p", bufs=2, space="PSUM") as ps_h2p,
            tc.tile_pool(name="ps_rep", bufs=2, space="PSUM") as ps_rep
            ,tc.tile_pool(name="ps_y", bufs=2, space="PSUM") as ps_y,
        ):
            s0 = consts.tile([64, 48], FP16)
            r8 = consts.tile([48, 128], FP32R)   # rows 0:16 and 32:48 both
                                                 # hold R8 (for base 0/32)
            m1a = consts.tile([128, 2, 48], FP32R)
            m1b = consts.tile([128, 2, 48], FP32R)
            r3_2 = consts.tile([48, 96], FP32R)
            m2b2 = consts.tile([98, 6], FP32R)
            for t, d in ((s0, c_s0),
                         (r8[0:16, :], c_r8), (r8[32:48, :], c_r8),
                         (m1a, c_m1a.rearrange("h k m -> k h m")),
                         (m1b, c_m1b.rearrange("h k m -> k h m")),
                         (r3_2, c_r3_2), (m2b2, c_m2b2)):
                nc.sync.dma_start(out=t[:], in_=d[:])

            relu = mybir.ActivationFunctionType.Relu

            # Experiment modes read one preloaded macro's data instead of
            # streaming from HBM (so the Tile framework sees every read tile
            # written exactly once, outside the timed loop).
            pre = {}
            if mode in ("compute", "mm"):
                NPc = G // 2
                pre["t0"] = consts.tile([64, NPc, F], FP16, name="t0c")
                pre["t1"] = consts.tile([128, 2, G * F], FP16, name="t1c")
                pre["t2"] = consts.tile([96, NPc, F], FP16, name="t2c")
                for h in range(2):
                    nc.sync.dma_start(
                        out=pre["t0"][32 * h:32 * h + 32, :, :],
                        in_=bass.AP(tensor=w[:].tensor, offset=h * F,
                                    ap=[[PIX, 32], [2 * F, NPc], [1, F]]))
                    nc.sync.dma_start(
                        out=pre["t2"][48 * h:48 * h + 48, :, :],
                        in_=bass.AP(tensor=w[:].tensor,
                                    offset=288 * PIX + h * F,
                                    ap=[[PIX, 48], [2 * F, NPc], [1, F]]))
                nc.sync.dma_start(
                    out=pre["t1"][:],
                    in_=w[32:288, 0:G * F].rearrange("(b p) x -> p b x", b=2))
            if mode == "mm":
                pre["h1"] = consts.tile([48, F], FP16, name="h1c")
                pre["prod"] = consts.tile([128, 2, F], FP16, name="prodc")
                pre["h2"] = consts.tile([48, F], FP16, name="h2c")
                pre["pcm"] = consts.tile([98, F], FP16, name="pcmc")
                nc.sync.dma_start(out=pre["h1"][:], in_=w[0:48, 0:F])
                nc.sync.dma_start(
                    out=pre["prod"][:],
                    in_=w[0:128, 0:2 * F].rearrange("p (b x) -> p b x", b=2))
                nc.sync.dma_start(out=pre["h2"][:], in_=w[48:96, 0:F])
                nc.sync.dma_start(out=pre["pcm"][:], in_=w[96:194, 0:F])

            def body():
                # Macro-tile of G*F pixels; inside, sub-chunks are processed
                # in PAIRS sharing PSUM banks at partition offsets 0/32 (both
                # legal matmul base partitions). This halves ACT op count and
                # fuses all of layer 2 (rep, products, reduce+bias) per pair.
                FM = G * F
                NP = G // 2     # pairs per macro
                for g in range(N_CHUNKS // G):
                    mp = slice(g * FM, (g + 1) * FM)
                    sls = [slice(i * F, (i + 1) * F) for i in range(G)]
                    psl = [slice(p * F, (p + 1) * F) for p in range(NP)]

                    t0m = loads.tile([64, NP, F], FP16, tag="t0", name="t0m",
                                     bufs=3)
                    t1m = loads.tile([128, 2, FM], FP16, tag="t1", name="t1m", bufs=3)
                    # t2 pair-stacked: partition h*48+ch, free (pair, x);
                    # h = parity of the sub-chunk within its pair
                    t2m = loads.tile([96, NP, F], FP16, tag="t2", name="t2m", bufs=3)
                    pcm = prods.tile([98, NP, F], FP32R, tag="pcm", name="pcm",
                                     bufs=3)
                    T0 = t0m if do_dma else pre["t0"]
                    T1 = t1m if do_dma else pre["t1"]
                    T2 = t2m if do_dma else pre["t2"]
                    if do_dma:
                        for h in range(2):
                            nc.sync.dma_start(
                                out=t0m[32 * h:32 * h + 32, :, :],
                                in_=bass.AP(tensor=w[:].tensor,
                                            offset=g * FM + h * F,
                                            ap=[[PIX, 32], [2 * F, NP], [1, F]]))
                        nc.sync.dma_start(
                            out=t1m[:],
                            in_=w[32:288, mp].rearrange("(b p) x -> p b x", b=2))
                        for h in range(2):
                            nc.sync.dma_start(
                                out=t2m[48 * h:48 * h + 48, :, :],
                                in_=bass.AP(tensor=w[:].tensor,
                                            offset=288 * PIX + g * FM + h * F,
                                            ap=[[PIX, 48], [2 * F, NP], [1, F]]))
                            nc.sync.dma_start(
                                out=pcm[96 + h:97 + h, :, :],
                                in_=bass.AP(tensor=wb[:].tensor,
                                            offset=g * FM + h * F,
                                            ap=[[2 * F, NP], [1, F]]))

                    h1pre = {}
                    for p in range(NP):
                        h1pre[p] = ps_sm16.tile([48, F], FP32, tag="sm16",
                                                name="h1pre")
                        if do_mm:
                            nc.tensor.matmul(h1pre[p][:], s0[:],
                                             T0[:, p, :],
                                             start=True, stop=True)
                    h1 = {}
                    for p in range(NP):
                        h1[p] = acts.tile([48, F], FP32R, tag="h1", name="h1")
                        if do_vec:
                            nc.scalar.activation(h1[p][:], h1pre[p][:], relu)
                    h1rep = {}
                    for p in range(NP):
                        for h in range(2):
                            h1rep[p, h] = ps_rep.tile([128, F], FP32,
                                                      tag="rep", name="h1rep")
                            if do_mm:
                                h1src = h1[p] if do_vec else pre["h1"]
                                nc.tensor.matmul(
                                    h1rep[p, h][:],
                                    r8[32 * h:32 * h + 16, :],
                                    h1src[32 * h:32 * h + 16, :],
                                    start=True, stop=True)
                    h1r_sb = {}
                    for p in range(NP):
                        h1r_sb[p] = acts.tile([128, F], FP16, tag="h1r",
                                              name="h1r", bufs=3)
                        if do_vec:
                            nc.scalar.copy(h1r_sb[p][:], h1rep[p, 1][:])
                    prodAB = {}
                    for p in range(NP):
                        for h in range(2):
                            prodAB[p, h] = prods.tile([128, 2, F], FP32R,
                                                      tag="prodAB",
                                                      name="prodAB", bufs=5)
                            if not do_vec:
                                continue
                            rp = h1rep[p, h]
                            rep2 = bass.AP(tensor=rp.tensor,
                                           offset=rp[:].offset,
                                           ap=[rp[:].ap[0], [0, 2],
                                               rp[:].ap[1]])
                            if h == 0:
                                nc.vector.tensor_mul(
                                    prodAB[p, h][:],
                                    T1[:, :, sls[2 * p + h]], rep2)
                            else:
                                sb = h1r_sb[p]
                                rep2s = bass.AP(tensor=sb.tensor,
                                                offset=sb[:].offset,
                                                ap=[sb[:].ap[0], [0, 2],
                                                    sb[:].ap[1]])
                                nc.gpsimd.tensor_mul(
                                    prodAB[p, h][:],
                                    T1[:, :, sls[2 * p + h]], rep2s)
                    h2pre = {}
                    for p in range(NP):
                        h2pre[p] = ps_h2p.tile([48, F], FP32, tag="h2p",
                                               name="h2pre")
                        if not do_mm:
                            continue
                        for h in range(2):
                            psrc = prodAB[p, h] if do_vec else pre["prod"]
                            nc.tensor.matmul(h2pre[p][:],
                                             m1a[:, h, :], psrc[:, 0, :],
                                             start=(h == 0), stop=False)
                            nc.tensor.matmul(h2pre[p][:],
                                             m1b[:, h, :], psrc[:, 1, :],
                                             start=False, stop=(h == 1))
                    h2 = {}
                    for p in range(NP):
                        h2[p] = acts.tile([48, F], FP32R, tag="h2", name="h2")
                        if do_vec:
                            nc.scalar.activation(h2[p][:], h2pre[p][:], relu)
                    h2rep = {}
                    for p in range(NP):
                        h2rep[p] = ps_rep.tile([96, F], FP32, tag="rep",
                                               name="h2rep")
                        if do_mm:
                            h2src = h2[p] if do_vec else pre["h2"]
                            nc.tensor.matmul(h2rep[p][:], r3_2[:], h2src[:],
                                             start=True, stop=True)
                    if do_vec:
                        for p in range(NP):
                            nc.vector.tensor_mul(pcm[0:96, p, :],
                                                 T2[:, p, :], h2rep[p][:])
                    y = {}
                    for p in range(NP):
                        y[p] = ps_y.tile([6, F], FP32, tag="y", name="y")
                        if do_mm:
                            if mode == "full":
                                nc.tensor.matmul(y[p][:], m2b2[:],
                                                 pcm[:, p, :],
                                                 start=True, stop=True)
                            elif mode == "compute":
                                nc.tensor.matmul(y[p][:], m2b2[0:96, :],
                                                 pcm[0:96, p, :],
                                                 start=True, stop=True)
                            else:
                                nc.tensor.matmul(y[p][:], m2b2[:],
                                                 pre["pcm"][:],
                                                 start=True, stop=True)
                    y_sb = outs.tile([6, NP, F], FP32, tag="ysb", name="ysb",
                                     bufs=2)
                    if do_vec:
                        for p in range(NP):
                            nc.scalar.copy(y_sb[:, p, :], y[p][:])
                    # y_sb partition h*3+j, free (pair, x) ->
                    # out[j, g*FM + pair*2F + h*F + x]
                    if mode == "full":
                        for h in range(2):
                            nc.sync.dma_start(
                                out=bass.AP(tensor=out[:].tensor,
                                            offset=g * FM + h * F,
                                            ap=[[PIX, 3], [2 * F, NP], [1, F]]),
                                in_=y_sb[3 * h:3 * h + 3, :, :])

            if repeat == 1:
                body()
            else:
                with tc.For_i(0, repeat, 1):
                    body()

    nc.compile()
    return nc


_NC_CACHE: dict[int, object] = {}


def _get_nc(repeat: int = 1):
    if repeat not in _NC_CACHE:
        _NC_CACHE[repeat] = build_nc(repeat)
    return _NC_CACHE[repeat]


def make_in_maps(weight: np.ndarray, coor: np.ndarray) -> list[dict]:
    mats = _const_mats(coor)
    in_maps = []
    for k in range(N_CORES):
        b, hh = k // 2, k % 2
        shard = np.ascontiguousarray(
            weight[b, :, hh * 128:(hh + 1) * 128, :].reshape(N_CH, PIX),
            dtype=np.float16)
        in_maps.append({"w": shard,
                        "wb": weight[b, 336, hh * 128:(hh + 1) * 128, :]
                        .reshape(1, PIX).astype(np.float32), **mats})
    return in_maps


def assemble_out(results: list[dict]) -> np.ndarray:
    out = np.empty((B, 3, H, W), np.float32)
    for k in range(N_CORES):
        b, hh = k // 2, k % 2
        out[b, :, hh * 128:(hh + 1) * 128, :] = results[k]["out"].reshape(3, 128, W)
    return out


def kernel(input: np.ndarray, weight: np.ndarray, coor: np.ndarray) -> np.ndarray:
    nc = _get_nc(1)
    in_maps = make_in_maps(np.asarray(weight), np.asarray(coor))
    res = run_bass_kernel_spmd(nc, in_maps, core_ids=list(range(N_CORES)))
    return assemble_out(res.results)



# revision 9
# speedup vs baseline: 1.0705x; 1.0684x over previous
"""Trainium2 Bass kernel for the per-pixel locally-connected MLP (dense_mlp).

Reference computation (per batch b, pixel (h,w)):
    x0 = coor (2-vector, shared by all pixels)
    h1 = relu(W0 @ x0)        W0 = weight[b, 0:32].reshape(16, 2)   per pixel
    h2 = relu(W1 @ h1)        W1 = weight[b, 32:288].reshape(16,16) per pixel
    y  = W2 @ h2 + bias       W2 = weight[b, 288:336].reshape(3,16), bias = weight[b,336]
Output: [4, 3, 256, 256] float32.

Sharding: 8 cores, core k handles batch k//2, image rows (k%2)*128:(k%2+1)*128
=> per-core weight shard [337, 32768] (channels x pixels); no cross-core comm.

Implementation notes:
- Channels live on SBUF partitions, pixels on the free axis, so every weight
  load is a wide contiguous DMA; weights are cast to fp16 on the host to halve
  HBM traffic (the kernel is purely memory-bound; rel err ~5e-4).
- The per-pixel matvecs are elementwise multiplies (VectorE) plus
  partition-axis reductions (TensorE matmuls against small host-built 0/1
  selection matrices; `coor` is folded into the first matmul's stationary
  matrix, the bias channel into the last one's moving operand). Matmul
  operands use float32r (TF32-like) for 4x PE throughput over fp32.
- Sub-chunks of 512 px (one PSUM bank) are processed in pairs that share PSUM
  banks: the pair's two halves land at partitions 0:16 / 32:48 of one bank via
  zero-padded stationaries accumulating at a base-0 dst, so each ScalarE
  relu / output-copy covers two chunks in one op.
- Work is emitted stage-major over 4096-px macro-tiles (3 merged HWDGE loads
  per macro + SWDGE bias/output DMAs on gpsimd) so the in-order engine queues
  pipeline across sub-chunks; pool buffer counts are sized to keep 2-3 macros
  in flight without deadlocking the Tile scheduler.
"""

import sys

for _p in ("/opt/trn_rl_repo", "/root/.axon_site/_ro/trn_rl_repo"):
    if _p not in sys.path:
        sys.path.append(_p)

import numpy as np

import concourse.bass as bass
import concourse.tile as tile
from concourse import bacc, mybir
from concourse.bass_utils import run_bass_kernel_spmd

# ---------------------------------------------------------------- constants
B, H, W = 4, 256, 256
N_CH = 337            # 32 (L0) + 256 (L1) + 48 (L2) + 1 (bias)
N_CORES = 8
PIX = (B * H * W) // N_CORES  # 32768 pixels per core
F = 512               # pixels per compute chunk (one PSUM bank of fp32)
N_CHUNKS = PIX // F

FP32 = mybir.dt.float32
FP32R = mybir.dt.float32r
FP16 = mybir.dt.float16


def _const_mats(coor: np.ndarray) -> dict[str, np.ndarray]:
    """Small stationary matrices for the TensorE reductions."""
    cx, cy = float(coor[0]), float(coor[1])
    # S0 for a pair-stacked moving operand t0 [64, F]: rows 0:32 are the
    # even chunk's L0 weights -> cols 0:16, rows 32:64 the odd chunk's ->
    # cols 32:48. One matmul produces both halves of h1pre.
    s0 = np.zeros((64, 48), np.float32)
    for h in range(2):
        for i in range(16):
            s0[32 * h + 2 * i, 32 * h + i] = cx
            s0[32 * h + 2 * i + 1, 32 * h + i] = cy
    r8 = np.zeros((16, 128), np.float32)      # h1rep[m] = h1[m % 16]
    for m in range(128):
        r8[m % 16, m] = 1.0
    m1a = np.zeros((2, 128, 48), np.float32)  # h2pre[j] += sum_i prodA[16j+i]
    m1b = np.zeros((2, 128, 48), np.float32)
    for h in range(2):
        for k in range(128):
            m1a[h, k, 32 * h + k // 16] = 1.0
            m1b[h, k, 32 * h + 8 + k // 16] = 1.0
    # pair-fused layer-2: moving operand is h2pair [48,F] with chunk A's h2
    # at rows 0:16 and chunk B's at rows 32:48 (rows 16:32 are junk)
    r3_2 = np.zeros((48, 96), np.float32)     # h2rep2[k] = h2(k//48)[k % 16]
    for k in range(96):
        r3_2[(0 if k < 48 else 32) + k % 16, k] = 1.0
    m2b2 = np.zeros((98, 6), np.float32)      # y[h*3+j] = sum prodC + bias
    for k in range(96):
        m2b2[k, (k // 48) * 3 + (k % 48) // 16] = 1.0
    m2b2[96, 0:3] = 1.0                       # bias row, even chunk
    m2b2[97, 3:6] = 1.0                       # bias row, odd chunk
    return {"s0": s0.astype(np.float16), "r8": r8, "m1a": m1a, "m1b": m1b,
            "r3_2": r3_2, "m2b2": m2b2}


def build_nc(repeat: int = 1, mode: str = "full"):
    """Build the per-core Bass program. `repeat` re-runs the whole kernel
    body sequentially (used only for differential HW timing). `mode` limits
    which instruction classes are emitted (timing experiments only):
    full | dma (loads/stores only) | compute (no HBM traffic) | mm
    (matmuls only)."""
    do_dma = mode in ("full", "dma")
    do_vec = mode in ("full", "compute")
    do_mm = mode in ("full", "compute", "mm")
    nc = bacc.Bacc(None, target_bir_lowering=False)

    w = nc.declare_dram_parameter("w", [N_CH, PIX], FP16, isOutput=False)
    wb = nc.declare_dram_parameter("wb", [1, PIX], FP32R, isOutput=False)
    out = nc.declare_dram_parameter("out", [3, PIX], FP32, isOutput=True)
    c_s0 = nc.declare_dram_parameter("s0", [64, 48], FP16, isOutput=False)
    c_r8 = nc.declare_dram_parameter("r8", [16, 128], FP32R, isOutput=False)
    c_m1a = nc.declare_dram_parameter("m1a", [2, 128, 48], FP32R, isOutput=False)
    c_m1b = nc.declare_dram_parameter("m1b", [2, 128, 48], FP32R, isOutput=False)
    c_r3_2 = nc.declare_dram_parameter("r3_2", [48, 96], FP32R, isOutput=False)
    c_m2b2 = nc.declare_dram_parameter("m2b2", [98, 6], FP32R, isOutput=False)

    G = 8                      # chunks per software-pipeline group
    with tile.TileContext(nc) as tc:
        with (
            tc.tile_pool(name="consts", bufs=1) as consts,
            tc.tile_pool(name="loads", bufs=2) as loads,
            tc.tile_pool(name="acts", bufs=4) as acts,
            tc.tile_pool(name="prods", bufs=3) as prods,
            tc.tile_pool(name="outs", bufs=2) as outs,
            tc.tile_pool(name="ps_sm16", bufs=2, space="PSUM") as ps_sm16,
            tc.tile_pool(name="ps_h2p", bufs=2, space="PSUM") as ps_h2p,
            tc.tile_pool(name="ps_rep", bufs=2, space="PSUM") as ps_rep
            ,tc.tile_pool(name="ps_y", bufs=2, space="PSUM") as ps_y,
        ):
            s0 = consts.tile([64, 48], FP16)
            r8 = consts.tile([48, 128], FP32R)   # rows 0:16 and 32:48 both
                                                 # hold R8 (for base 0/32)
            m1a = consts.tile([128, 2, 48], FP32R)
            m1b = consts.tile([128, 2, 48], FP32R)
            r3_2 = consts.tile([48, 96], FP32R)
            m2b2 = consts.tile([98, 6], FP32R)
            for t, d in ((s0, c_s0),
                         (r8[0:16, :], c_r8), (r8[32:48, :], c_r8),
                         (m1a, c_m1a.rearrange("h k m -> k h m")),
                         (m1b, c_m1b.rearrange("h k m -> k h m")),
                         (r3_2, c_r3_2), (m2b2, c_m2b2)):
                nc.sync.dma_start(out=t[:], in_=d[:])

            relu = mybir.ActivationFunctionType.Relu

            # Experiment modes read one preloaded macro's data instead of
            # streaming from HBM (so the Tile framework sees every read tile
            # written exactly once, outside the timed loop).
            pre = {}
            if mode in ("compute", "mm"):
                NPc = G // 2
                pre["t0"] = consts.tile([64, NPc, F], FP16, name="t0c")
                pre["t1"] = consts.tile([128, 2, G * F], FP16, name="t1c")
                pre["t2"] = consts.tile([96, NPc, F], FP16, name="t2c")
                for h in range(2):
                    nc.sync.dma_start(
                        out=pre["t0"][32 * h:32 * h + 32, :, :],
                        in_=bass.AP(tensor=w[:].tensor, offset=h * F,
                                    ap=[[PIX, 32], [2 * F, NPc], [1, F]]))
                    nc.sync.dma_start(
                        out=pre["t2"][48 * h:48 * h + 48, :, :],
                        in_=bass.AP(tensor=w[:].tensor,
                                    offset=288 * PIX + h * F,
                                    ap=[[PIX, 48], [2 * F, NPc], [1, F]]))
                nc.sync.dma_start(
                    out=pre["t1"][:],
                    in_=w[32:288, 0:G * F].rearrange("(b p) x -> p b x", b=2))
            if mode == "mm":
                # fp32r moving operands (stationaries are fp32r; fp16/fp32r
                # mixes are rejected by the compiler), junk-filled from wb.
                pre["h1"] = consts.tile([48, F], FP32R, name="h1c")
                pre["prod"] = consts.tile([128, 2, F], FP32R, name="prodc")
                pre["h2"] = consts.tile([48, F], FP32R, name="h2c")
                pre["pcm"] = consts.tile([98, F], FP32R, name="pcmc")
                wbt = wb[:].tensor
                nc.sync.dma_start(out=pre["h1"][:], in_=bass.AP(
                    tensor=wbt, offset=0, ap=[[64, 48], [1, F]]))
                nc.sync.dma_start(out=pre["prod"][:], in_=bass.AP(
                    tensor=wbt, offset=0, ap=[[128, 128], [F, 2], [1, F]]))
                nc.sync.dma_start(out=pre["h2"][:], in_=bass.AP(
                    tensor=wbt, offset=0, ap=[[64, 48], [1, F]]))
                nc.sync.dma_start(out=pre["pcm"][:], in_=bass.AP(
                    tensor=wbt, offset=0, ap=[[256, 98], [1, F]]))

            def body():
                # Macro-tile of G*F pixels; inside, sub-chunks are processed
                # in PAIRS sharing PSUM banks at partition offsets 0/32 (both
                # legal matmul base partitions). This halves ACT op count and
                # fuses all of layer 2 (rep, products, reduce+bias) per pair.
                FM = G * F
                NP = G // 2     # pairs per macro
                for g in range(N_CHUNKS // G):
                    mp = slice(g * FM, (g + 1) * FM)
                    sls = [slice(i * F, (i + 1) * F) for i in range(G)]
                    psl = [slice(p * F, (p + 1) * F) for p in range(NP)]

                    t0m = loads.tile([64, NP, F], FP16, tag="t0", name="t0m",
                                     bufs=3)
                    t1m = loads.tile([128, 2, FM], FP16, tag="t1", name="t1m", bufs=3)
                    # t2 pair-stacked: partition h*48+ch, free (pair, x);
                    # h = parity of the sub-chunk within its pair
                    t2m = loads.tile([96, NP, F], FP16, tag="t2", name="t2m", bufs=3)
                    pcm = prods.tile([98, NP, F], FP32R, tag="pcm", name="pcm",
                                     bufs=3)
                    T0 = t0m if do_dma else pre["t0"]
                    T1 = t1m if do_dma else pre["t1"]
                    T2 = t2m if do_dma else pre["t2"]
                    if do_dma:
                        for h in range(2):
                            nc.sync.dma_start(
                                out=t0m[32 * h:32 * h + 32, :, :],
                                in_=bass.AP(tensor=w[:].tensor,
                                            offset=g * FM + h * F,
                                            ap=[[PIX, 32], [2 * F, NP], [1, F]]))
                        nc.sync.dma_start(
                            out=t1m[:],
                            in_=w[32:288, mp].rearrange("(b p) x -> p b x", b=2))
                        for h in range(2):
                            nc.sync.dma_start(
                                out=t2m[48 * h:48 * h + 48, :, :],
                                in_=bass.AP(tensor=w[:].tensor,
                                            offset=288 * PIX + g * FM + h * F,
                                            ap=[[PIX, 48], [2 * F, NP], [1, F]]))
                            nc.sync.dma_start(
                                out=pcm[96 + h:97 + h, :, :],
                                in_=bass.AP(tensor=wb[:].tensor,
                                            offset=g * FM + h * F,
                                            ap=[[2 * F, NP], [1, F]]))

                    h1pre = {}
                    for p in range(NP):
                        h1pre[p] = ps_sm16.tile([48, F], FP32, tag="sm16",
                                                name="h1pre")
                        if do_mm:
                            nc.tensor.matmul(h1pre[p][:], s0[:],
                                             T0[:, p, :],
                                             start=True, stop=True)
                    h1 = {}
                    for p in range(NP):
                        h1[p] = acts.tile([48, F], FP32R, tag="h1", name="h1")
                        if do_vec:
                            nc.scalar.activation(h1[p][:], h1pre[p][:], relu)
                    h1rep = {}
                    for p in range(NP):
                        for h in range(2):
                            h1rep[p, h] = ps_rep.tile([128, F], FP32,
                                                      tag="rep", name="h1rep")
                            if do_mm:
                                h1src = h1[p] if do_vec else pre["h1"]
                                nc.tensor.matmul(
                                    h1rep[p, h][:],
                                    r8[32 * h:32 * h + 16, :],
                                    h1src[32 * h:32 * h + 16, :],
                                    start=True, stop=True)
                    h1r_sb = {}
                    for p in range(NP):
                        h1r_sb[p] = acts.tile([128, F], FP16, tag="h1r",
                                              name="h1r", bufs=3)
                        if do_vec:
                            nc.scalar.copy(h1r_sb[p][:], h1rep[p, 1][:])
                    prodAB = {}
                    for p in range(NP):
                        for h in range(2):
                            prodAB[p, h] = prods.tile([128, 2, F], FP32R,
                                                      tag="prodAB",
                                                      name="prodAB", bufs=5)
                            if not do_vec:
                                continue
                            rp = h1rep[p, h]
                            rep2 = bass.AP(tensor=rp.tensor,
                                           offset=rp[:].offset,
                                           ap=[rp[:].ap[0], [0, 2],
                                               rp[:].ap[1]])
                            if h == 0:
                                nc.vector.tensor_mul(
                                    prodAB[p, h][:],
                                    T1[:, :, sls[2 * p + h]], rep2)
                            else:
                                sb = h1r_sb[p]
                                rep2s = bass.AP(tensor=sb.tensor,
                                                offset=sb[:].offset,
                                                ap=[sb[:].ap[0], [0, 2],
                                                    sb[:].ap[1]])
                                nc.gpsimd.tensor_mul(
                                    prodAB[p, h][:],
                                    T1[:, :, sls[2 * p + h]], rep2s)
                    h2pre = {}
                    for p in range(NP):
                        h2pre[p] = ps_h2p.tile([48, F], FP32, tag="h2p",
                                               name="h2pre")
                        if not do_mm:
                            continue
                        for h in range(2):
                            psrc = prodAB[p, h] if do_vec else pre["prod"]
                            nc.tensor.matmul(h2pre[p][:],
                                             m1a[:, h, :], psrc[:, 0, :],
                                             start=(h == 0), stop=False)
                            nc.tensor.matmul(h2pre[p][:],
                                             m1b[:, h, :], psrc[:, 1, :],
                                             start=False, stop=(h == 1))
                    h2 = {}
                    for p in range(NP):
                        h2[p] = acts.tile([48, F], FP32R, tag="h2", name="h2")
                        if do_vec:
                            nc.scalar.activation(h2[p][:], h2pre[p][:], relu)
                    h2rep = {}
                    for p in range(NP):
                        h2rep[p] = ps_rep.tile([96, F], FP32, tag="rep",
                                               name="h2rep")
                        if do_mm:
                            h2src = h2[p] if do_vec else pre["h2"]
                            nc.tensor.matmul(h2rep[p][:], r3_2[:], h2src[:],
                                             start=True, stop=True)
                    if do_vec:
                        for p in range(NP):
                            nc.vector.tensor_mul(pcm[0:96, p, :],
                                                 T2[:, p, :], h2rep[p][:])
                    y = {}
                    for p in range(NP):
                        y[p] = ps_y.tile([6, F], FP32, tag="y", name="y")
                        if do_mm:
                            if mode == "full":
                                nc.tensor.matmul(y[p][:], m2b2[:],
                                                 pcm[:, p, :],
                                                 start=True, stop=True)
                            elif mode == "compute":
                                nc.tensor.matmul(y[p][:], m2b2[0:96, :],
                                                 pcm[0:96, p, :],
                                                 start=True, stop=True)
                            else:
                                nc.tensor.matmul(y[p][:], m2b2[:],
                                                 pre["pcm"][:],
                                                 start=True, stop=True)
                    y_sb = outs.tile([6, NP, F], FP32, tag="ysb", name="ysb",
                                     bufs=2)
                    if do_vec:
                        for p in range(NP):
                            nc.scalar.copy(y_sb[:, p, :], y[p][:])
                    # y_sb partition h*3+j, free (pair, x) ->
                    # out[j, g*FM + pair*2F + h*F + x]
                    if mode == "full":
                        for h in range(2):
                            nc.sync.dma_start(
                                out=bass.AP(tensor=out[:].tensor,
                                            offset=g * FM + h * F,
                                            ap=[[PIX, 3], [2 * F, NP], [1, F]]),
                                in_=y_sb[3 * h:3 * h + 3, :, :])

            if repeat == 1:
                body()
            else:
                with tc.For_i(0, repeat, 1):
                    body()

    nc.compile()
    return nc


_NC_CACHE: dict[int, object] = {}


def _get_nc(repeat: int = 1):
    if repeat not in _NC_CACHE:
        _NC_CACHE[repeat] = build_nc(repeat)
    return _NC_CACHE[repeat]


def make_in_maps(weight: np.ndarray, coor: np.ndarray) -> list[dict]:
    mats = _const_mats(coor)
    in_maps = []
    for k in range(N_CORES):
        b, hh = k // 2, k % 2
        shard = np.ascontiguousarray(
            weight[b, :, hh * 128:(hh + 1) * 128, :].reshape(N_CH, PIX),
            dtype=np.float16)
        in_maps.append({"w": shard,
                        "wb": weight[b, 336, hh * 128:(hh + 1) * 128, :]
                        .reshape(1, PIX).astype(np.float32), **mats})
    return in_maps


def assemble_out(results: list[dict]) -> np.ndarray:
    out = np.empty((B, 3, H, W), np.float32)
    for k in range(N_CORES):
        b, hh = k // 2, k % 2
        out[b, :, hh * 128:(hh + 1) * 128, :] = results[k]["out"].reshape(3, 128, W)
    return out


def kernel(input: np.ndarray, weight: np.ndarray, coor: np.ndarray) -> np.ndarray:
    nc = _get_nc(1)
    in_maps = make_in_maps(np.asarray(weight), np.asarray(coor))
    res = run_bass_kernel_spmd(nc, in_maps, core_ids=list(range(N_CORES)))
    return assemble_out(res.results)



# revision 10
# speedup vs baseline: 1.2234x; 1.1428x over previous
"""Trainium2 Bass kernel v2 for the per-pixel locally-connected MLP.

Reference computation (per batch b, pixel (h,w)):
    x0 = coor (2-vector, shared by all pixels)
    h1 = relu(W0 @ x0)        W0 = weight[b, 0:32].reshape(16, 2)   per pixel
    h2 = relu(W1 @ h1)        W1 = weight[b, 32:288].reshape(16,16) per pixel
    y  = W2 @ h2 + bias       W2 = weight[b, 288:336].reshape(3,16), bias = weight[b,336]
Output: [4, 3, 256, 256] float32.

Sharding: 8 cores, core k handles batch k//2, image rows (k%2)*128:(k%2+1)*128
=> per-core weight shard [337, 32768] fp16 (channels x pixels); no comm.

v2 design (vs v1): the replication of h1 (x8, against W1's 256 rows) and of
h2 (x3, against W2's 48 rows) is folded into the stationary matrices of the
preceding matmuls, so the separate replication matmuls and PSUM->SBUF
copies disappear.  Each PSUM->SBUF move is a fused relu+cast producing
fp16, which lets VectorE run its elementwise multiplies in 2x packed mode
and lets GpSimd (no PSUM access) read operands directly.  The 4 per-chunk
L0 matmuls of a quad sit in distinct 32-row strips (tile_position) and run
concurrently; the 4 y matmuls of a macro are column-tiled into one PSUM
bank so one ScalarE copy covers 4096 pixels.  The output leaves the core
in a [24, 8, F] scratch layout (4 contiguous-partition stores per macro —
scattered-partition store APs are illegal) and the host untangles it.
"""

import sys

for _p in ("/opt/trn_rl_repo", "/root/.axon_site/_ro/trn_rl_repo"):
    if _p not in sys.path:
        sys.path.append(_p)

import numpy as np

import concourse.bass as bass
import concourse.tile as tile
from concourse import bacc, mybir
from concourse.bass_utils import run_bass_kernel_spmd

# ---------------------------------------------------------------- constants
B, H, W = 4, 256, 256
N_CH = 337            # 32 (L0) + 256 (L1) + 48 (L2) + 1 (bias)
N_CORES = 8
PIX = (B * H * W) // N_CORES  # 32768 pixels per core
F = 512               # pixels per compute chunk (one PSUM bank of fp32)
N_CHUNKS = PIX // F   # 64

FP32 = mybir.dt.float32
FP16 = mybir.dt.float16

# Optimized DRAM layout of the weight shard (host-built in make_in_maps):
#   A [g][p][b][x]   W1 rows as t1m partitions, big runs   (256*PIX elems)
#   B [g][c][ch][q][x]  W0 for t0q                          (32*PIX)
#   C [g][h][ch2][p][x] W2 for t2m                          (48*PIX)
#   D [g][h][p][x]   bias rows                              (PIX)
OFF_A = 0
OFF_B = OFF_A + 256 * PIX
OFF_C = OFF_B + 32 * PIX
OFF_D = OFF_C + 48 * PIX
TOT_W = OFF_D + PIX


def _geom():
    """(G, FM, NMAC, NQ, NP) from CFG["G"]: chunks/macro, px/macro,
    macros, quads/macro, pairs/macro.  Chunk k of a macro = quad q=k%NQ,
    quad-position c=k//NQ; pair p = chunks (2p, 2p+1), parity h=k%2."""
    G = CFG["G"]
    return G, G * F, N_CHUNKS // G, G // 4, G // 2

# Engine/buffer tuning knobs (sweepable via exp2 configs).
CFG = {
    # L1 product engine per chunk k%4: 'v' (DVE), 'g' (gpsimd), or
    # 'gv' (plane 0 gpsimd, plane 1 DVE)
    "l1": {0: "v", 1: "v", 2: "v", 3: "v"},
    # relu-L2 route per pair: 'a' (ACT psum->sbuf), 'v' (DVE psum->sbuf),
    # 'vp' (DVE in-place in PSUM; pcm mul then reads PSUM directly)
    "relu2": {0: "a", 1: "v", 2: "vp", 3: "vp"},
    # L2 product engine per pair ('vp' is implied by relu2 'vp')
    "l2": {0: "v", 1: "v", 2: "v", 3: "v"},
    "ycopy": "v",
    "prod_bufs": 12,
    "h1r_bufs": 6,
    "h2r_bufs": 6,
    "pcm_bufs": 4,
    "load_bufs": 4,
    "ysb_bufs": 3,
    "psl0_bufs": 2,
    "psh2p_bufs": 3,
    "psy_bufs": 1,
    "G": 8,
    "swpipe": 2,      # 0=monolithic, 1=back-half lag 1, 2=three-phase
    "hp_relu1": None,   # high_priority offset for relu1 (None = off)
    "hp_loads": None,   # high_priority offset for weight loads
}


def _const_mats(coor: np.ndarray) -> dict[str, np.ndarray]:
    cx, cy = float(coor[0]), float(coor[1])
    # s0r4: L0 stationary producing h1pre replicated x8 on 128 partitions.
    # Quad-position c uses rows 32c:32c+32 (its own PE row strip); the block
    # is identical for each c.  Row 2i+d (channel of W0), col m -> coor[d]
    # when m % 16 == i.
    blk = np.zeros((32, 128), np.float32)
    for i in range(16):
        for m in range(128):
            if m % 16 == i:
                blk[2 * i, m] = cx
                blk[2 * i + 1, m] = cy
    s0r4 = np.tile(blk, (4, 1)).astype(np.float16)
    # m1ra/b: L1 reduction stationaries with the x3 L2 replication fused in.
    # prod plane A partition k holds W1 row k = 16j+i (j=k//16<8); plane B
    # row 128+k (j=8+k//16).  Output col (pair-packed) 48h + 16o + j.
    m1ra = np.zeros((128, 2, 96), np.float32)
    m1rb = np.zeros((128, 2, 96), np.float32)
    for k in range(128):
        for h in range(2):
            for o in range(3):
                m1ra[k, h, 48 * h + 16 * o + k // 16] = 1.0
                m1rb[k, h, 48 * h + 16 * o + 8 + k // 16] = 1.0
    # y reduction: pcm row k = 48h + 16o + j -> output 3h + o; bias rows.
    m2b2 = np.zeros((98, 6), np.float32)
    for k in range(96):
        m2b2[k, (k // 48) * 3 + (k % 48) // 16] = 1.0
    m2b2[96, 0:3] = 1.0
    m2b2[97, 3:6] = 1.0
    return {"s0r4": s0r4,
            "m1ra": m1ra.astype(np.float16), "m1rb": m1rb.astype(np.float16),
            "m2b2": m2b2.astype(np.float16)}


def build_nc(repeat: int = 1, mode: str = "full"):
    """mode: full | dma | compute | mm | vec (timing experiments)."""
    do_dma = mode in ("full", "dma")
    do_vec = mode in ("full", "compute", "vec")
    do_mm = mode in ("full", "compute", "mm")
    G, FM, NMAC, NQ, NP = _geom()
    SB = NQ * F
    nc = bacc.Bacc(None, target_bir_lowering=False)

    w = nc.declare_dram_parameter("w", [1, TOT_W], FP16, isOutput=False)
    # scratch output layout: row 6p+3h+j, macro g, x  ->  host untangles
    out = nc.declare_dram_parameter("out", [6 * NP, NMAC * F], FP32,
                                    isOutput=True)
    c_s0 = nc.declare_dram_parameter("s0r4", [128, 128], FP16, isOutput=False)
    c_m1a = nc.declare_dram_parameter("m1ra", [128, 2, 96], FP16,
                                      isOutput=False)
    c_m1b = nc.declare_dram_parameter("m1rb", [128, 2, 96], FP16,
                                      isOutput=False)
    c_m2 = nc.declare_dram_parameter("m2b2", [98, 6], FP16, isOutput=False)

    relu = mybir.ActivationFunctionType.Relu
    wt = w[:].tensor

    with tile.TileContext(nc) as tc:
        with (
            tc.tile_pool(name="consts", bufs=1) as consts,
            tc.tile_pool(name="loads", bufs=2) as loads,
            tc.tile_pool(name="acts", bufs=4) as acts,
            tc.tile_pool(name="prods", bufs=3) as prods,
            tc.tile_pool(name="outs", bufs=2) as outs,
            tc.tile_pool(name="ps_l0", bufs=CFG["psl0_bufs"],
                         space="PSUM") as ps_l0,
            tc.tile_pool(name="ps_h2p", bufs=CFG["psh2p_bufs"],
                         space="PSUM") as ps_h2p,
            tc.tile_pool(name="ps_y", bufs=CFG["psy_bufs"],
                         space="PSUM") as ps_y,
        ):
            s0r4 = consts.tile([128, 128], FP16)
            m1ra = consts.tile([128, 2, 96], FP16)
            m1rb = consts.tile([128, 2, 96], FP16)
            m2b2 = consts.tile([98, 6], FP16)
            for t, d in ((s0r4, c_s0), (m1ra, c_m1a), (m1rb, c_m1b),
                         (m2b2, c_m2)):
                nc.sync.dma_start(out=t[:], in_=d[:])

            pre = {}
            if mode in ("compute", "mm", "vec"):
                pre["t0"] = consts.tile([128, NQ, F], FP16, name="t0c")
                pre["t1"] = consts.tile([128, 2, FM], FP16, name="t1c")
                pre["t2"] = consts.tile([96, NP, F], FP16, name="t2c")
                pre["pc"] = consts.tile([98, NP, F], FP16, name="pcc")
                nc.sync.dma_start(
                    out=pre["t0"][:],
                    in_=bass.AP(tensor=wt, offset=OFF_B,
                                ap=[[32 * SB, 4], [SB, 32], [1, SB]]))
                nc.sync.dma_start(
                    out=pre["t1"][:],
                    in_=bass.AP(tensor=wt, offset=OFF_A,
                                ap=[[2 * FM, 128], [FM, 2], [1, FM]]))
                nc.sync.dma_start(
                    out=pre["t2"][:],
                    in_=bass.AP(tensor=wt, offset=OFF_C,
                                ap=[[48 * NP * F, 2], [NP * F, 48],
                                    [1, NP * F]]))
                nc.sync.dma_start(
                    out=pre["pc"][96:98, :, :],
                    in_=bass.AP(tensor=wt, offset=OFF_D,
                                ap=[[NP * F, 2], [1, NP * F]]))
                nc.sync.dma_start(
                    out=pre["pc"][0:96, :, :],
                    in_=bass.AP(tensor=wt, offset=0,
                                ap=[[196, 96], [F, 4], [1, F]]))
            if mode == "vec":
                pre["l0"] = ps_l0.tile([128, 2, F], FP32, name="l0c")
                pre["h2p"] = ps_h2p.tile([96, F], FP32, name="h2pc")
                pre["yb"] = ps_y.tile([128, F], FP32, name="ybc")
                nc.vector.memset(pre["l0"][:], 0.25)
                nc.vector.memset(pre["h2p"][:], 0.25)
                nc.vector.memset(pre["yb"][:], 0.25)
            if mode == "mm":
                pre["h1r"] = consts.tile([128, 2, F], FP16, name="h1rc")
                pre["h2r"] = consts.tile([96, F], FP16, name="h2rc")
                pre["prod"] = consts.tile([128, 2, F], FP16, name="prodc")
                nc.sync.dma_start(out=pre["h1r"][:], in_=bass.AP(
                    tensor=wt, offset=0, ap=[[128, 128], [F, 2], [1, F]]))
                nc.sync.dma_start(out=pre["h2r"][:], in_=bass.AP(
                    tensor=wt, offset=0, ap=[[64, 96], [1, F]]))
                nc.sync.dma_start(out=pre["prod"][:], in_=bass.AP(
                    tensor=wt, offset=0, ap=[[128, 128], [F, 2], [1, F]]))

            def front(g):
                    sls = [slice(i * F, (i + 1) * F) for i in range(G)]

                    t0q = loads.tile([128, NQ, F], FP16, tag="t0",
                                     name="t0q", bufs=CFG["load_bufs"])
                    t1m = loads.tile([128, 2, FM], FP16, tag="t1", name="t1m",
                                     bufs=CFG["load_bufs"])
                    t2m = loads.tile([96, NP, F], FP16, tag="t2", name="t2m",
                                     bufs=CFG["load_bufs"])
                    pcm = prods.tile([98, NP, F], FP16, tag="pcm", name="pcm",
                                     bufs=CFG["pcm_bufs"])
                    T0 = t0q if do_dma else pre["t0"]
                    T1 = t1m if do_dma else pre["t1"]
                    T2 = t2m if do_dma else pre["t2"]
                    import contextlib
                    _ol = CFG["hp_loads"]
                    hp_l = (tc.high_priority(None if _ol == "front" else _ol)
                            if _ol is not None
                            else contextlib.nullcontext())
                    if do_dma:
                      with hp_l:
                        # t0q partition 32c+ch = W0 channel ch, chunk NQ*c+q
                        nc.sync.dma_start(
                            out=t0q[:],
                            in_=bass.AP(tensor=wt,
                                        offset=OFF_B + g * 128 * SB,
                                        ap=[[32 * SB, 4], [SB, 32],
                                            [1, SB]]))
                        nc.sync.dma_start(
                            out=t1m[:],
                            in_=bass.AP(tensor=wt,
                                        offset=OFF_A + g * 256 * FM,
                                        ap=[[2 * FM, 128], [FM, 2],
                                            [1, FM]]))
                        # t2m partition 48h+ch2 = W2 channel ch2, pair p
                        nc.sync.dma_start(
                            out=t2m[:],
                            in_=bass.AP(tensor=wt,
                                        offset=OFF_C + g * 48 * FM,
                                        ap=[[48 * NP * F, 2], [NP * F, 48],
                                            [1, NP * F]]))
                        nc.sync.dma_start(
                            out=pcm[96:98, :, :],
                            in_=bass.AP(tensor=wt,
                                        offset=OFF_D + g * FM,
                                        ap=[[NP * F, 2], [1, NP * F]]))

                    # ---- L0: replicated h1pre, 4 row-strip matmuls/quad
                    h1r = {}        # h1r[q, t] fp16 [128, 2, F]
                    for q in range(NQ):
                        for t in range(2):
                            if mode == "vec":
                                l0 = pre["l0"]
                            else:
                                l0 = ps_l0.tile([128, 2, F], FP32, tag="l0",
                                                name="l0")
                            if do_mm:
                                for cc in range(2):
                                    c = 2 * t + cc
                                    nc.tensor.matmul(
                                        l0[:, cc, :],
                                        s0r4[32 * c:32 * c + 32, :],
                                        T0[32 * c:32 * c + 32, q, :],
                                        start=True, stop=True,
                                        tile_position=(32 * c, 0))
                            hr = acts.tile([128, 2, F], FP16, tag="h1r",
                                           name="h1r", bufs=CFG["h1r_bufs"])
                            if do_vec:
                                if CFG["hp_relu1"] is not None:
                                    _o = CFG["hp_relu1"]
                                    with tc.high_priority(
                                            None if _o == "front" else _o):
                                        nc.scalar.activation(hr[:], l0[:],
                                                             relu)
                                else:
                                    nc.scalar.activation(hr[:], l0[:], relu)
                            h1r[q, t] = hr

                    # ---- L1 products (fp16): DVE 2x / gpsimd split
                    prodAB = {}
                    for k in range(G):
                        q, c = k % NQ, k // NQ
                        t, cc = c // 2, c % 2
                        prodAB[k] = prods.tile([128, 2, F], FP16,
                                               tag="prod", name="prod",
                                               bufs=CFG["prod_bufs"])
                        if not do_vec:
                            continue
                        src = h1r[q, t] if mode != "mm" else pre["h1r"]
                        sl = src[:, cc, :]
                        rep2 = bass.AP(tensor=src.tensor, offset=sl.offset,
                                       ap=[sl.ap[0], [0, 2], sl.ap[1]])
                        tsl = T1[:, :, sls[k]]
                        eng = CFG["l1"][k % 4]
                        if eng == "g":
                            nc.gpsimd.tensor_mul(prodAB[k][:], tsl, rep2)
                        elif eng == "gv":
                            nc.gpsimd.tensor_mul(prodAB[k][:, 0, :],
                                                 T1[:, 0, sls[k]],
                                                 src[:, cc, :])
                            nc.vector.tensor_mul(prodAB[k][:, 1, :],
                                                 T1[:, 1, sls[k]],
                                                 src[:, cc, :])
                        else:
                            nc.vector.tensor_mul(prodAB[k][:], tsl, rep2)

                    return {"g": g, "prodAB": prodAB, "pcm": pcm,
                            "T2": T2}

            def mid(ctx):
                    g = ctx["g"]
                    prodAB, pcm, T2 = ctx["prodAB"], ctx["pcm"], ctx["T2"]
                    # ---- L1 reduction with fused x3 replication, relu
                    h2r = {}
                    for p in range(NP):
                        if mode == "vec":
                            h2p = pre["h2p"]
                        else:
                            h2p = ps_h2p.tile([96, F], FP32, tag="h2p",
                                              name="h2p")
                        if do_mm:
                            for h in range(2):
                                ps = (prodAB[2 * p + h] if mode != "mm"
                                      else pre["prod"])
                                nc.tensor.matmul(h2p[:], m1ra[:, h, :],
                                                 ps[:, 0, :],
                                                 start=(h == 0), stop=False)
                                nc.tensor.matmul(h2p[:], m1rb[:, h, :],
                                                 ps[:, 1, :],
                                                 start=False, stop=(h == 1))
                        route = CFG["relu2"][p % 4]
                        if route == "vp":
                            if do_vec and mode != "vec":
                                nc.vector.tensor_scalar_max(h2p[:], h2p[:],
                                                            0.0)
                            h2r[p] = h2p
                        else:
                            hr2 = acts.tile([96, F], FP16, tag="h2r",
                                            name="h2r",
                                            bufs=CFG["h2r_bufs"])
                            if do_vec:
                                if route == "a":
                                    nc.scalar.activation(hr2[:], h2p[:],
                                                         relu)
                                else:
                                    nc.vector.tensor_scalar_max(hr2[:],
                                                                h2p[:], 0.0)
                            h2r[p] = hr2

                    # ---- L2 products
                    if do_vec:
                        for p in range(NP):
                            src = h2r[p] if mode != "mm" else pre["h2r"]
                            e = (nc.gpsimd if CFG["l2"][p % 4] == "g"
                                 else nc.vector)
                            e.tensor_mul(pcm[0:96, p, :],
                                         T2[:, p, :], src[:])
                            del src
                    return ctx

            def tail(ctx):
                    g = ctx["g"]
                    pcm = ctx["pcm"]
                    # ---- y: 4 column-tiled matmuls into one bank
                    if mode == "vec":
                        yb = pre["yb"]
                    else:
                        yb = ps_y.tile([128, F], FP32, tag="y", name="yb")
                    if do_mm:
                        for p in range(NP):
                            if mode == "compute":
                                # bias rows aren't loaded in compute mode
                                mv, st = pcm[0:96, p, :], m2b2[0:96, :]
                            elif mode == "mm":
                                mv, st = pre["pc"][:, p, :], m2b2[:]
                            else:
                                mv, st = pcm[:, p, :], m2b2[:]
                            nc.tensor.matmul(yb[32 * p:32 * p + 6, :],
                                             st, mv,
                                             start=True, stop=True,
                                             tile_position=(0, 32 * p))
                    ysb = outs.tile([104, F], FP32, tag="ysb", name="ysb",
                                    bufs=CFG["ysb_bufs"])
                    ytop = 32 * (NP - 1) + 6
                    if do_vec:
                        if CFG["ycopy"] == "a":
                            nc.scalar.copy(ysb[0:ytop, :], yb[0:ytop, :])
                        else:
                            nc.vector.tensor_copy(ysb[0:ytop, :],
                                                  yb[0:ytop, :])
                    # store: ysb rows 32p+3h+j -> out row 6p+3h+j, col g*F+x
                    if mode == "full":
                        for p in range(NP):
                            nc.sync.dma_start(
                                out=bass.AP(tensor=out[:].tensor,
                                            offset=6 * p * NMAC * F + g * F,
                                            ap=[[NMAC * F, 6], [1, F]]),
                                in_=ysb[32 * p:32 * p + 6, :])

            def body():
                # Chunk k of a macro covers pixels [g*FM + k*F, ...+F).
                # Quad q = chunks {k: k%NQ == q}, quad-position c = k//NQ.
                # Pair p = chunks (2p, 2p+1), parity h = k%2.
                sw = int(CFG["swpipe"])

                def run_mid(c):
                    mid(c)
                    c["_m"] = True

                q = []
                for g in range(NMAC):
                    q.append(front(g))
                    if sw == 0:
                        c = q.pop()
                        run_mid(c)
                        tail(c)
                    elif sw == 1:
                        if len(q) > 1:
                            c = q.pop(0)
                            run_mid(c)
                            tail(c)
                    else:
                        if len(q) >= 2:
                            run_mid(q[-2])
                        if len(q) >= 3:
                            tail(q.pop(0))
                for c in q:
                    if not c.get("_m"):
                        run_mid(c)
                    tail(c)

            if repeat == 1:
                body()
            else:
                with tc.For_i(0, repeat, 1):
                    body()

    nc.compile()
    return nc


_NC_CACHE: dict[int, object] = {}


def _get_nc(repeat: int = 1):
    if repeat not in _NC_CACHE:
        _NC_CACHE[repeat] = build_nc(repeat)
    return _NC_CACHE[repeat]


def _relayout(shard: np.ndarray) -> np.ndarray:
    """[337, PIX] fp16 -> flat optimized-layout [1, TOT_W]."""
    G, FM, NMAC, NQ, NP = _geom()
    A = shard[32:288].reshape(2, 128, NMAC, FM).transpose(2, 1, 0, 3)
    Bb = shard[0:32].reshape(32, NMAC, 4, NQ, F).transpose(1, 2, 0, 3, 4)
    C = shard[288:336].reshape(48, NMAC, NP, 2, F).transpose(1, 3, 0, 2, 4)
    D = shard[336].reshape(NMAC, NP, 2, F).transpose(0, 2, 1, 3)
    return np.concatenate(
        [A.ravel(), Bb.ravel(), C.ravel(), D.ravel()]).reshape(1, TOT_W)


def make_in_maps(weight: np.ndarray, coor: np.ndarray) -> list[dict]:
    mats = _const_mats(coor)
    in_maps = []
    for k in range(N_CORES):
        b, hh = k // 2, k % 2
        shard = np.ascontiguousarray(
            weight[b, :, hh * 128:(hh + 1) * 128, :].reshape(N_CH, PIX),
            dtype=np.float16)
        in_maps.append({"w": _relayout(shard), "worig": shard, **mats})
    return in_maps


def assemble_out(results: list[dict]) -> np.ndarray:
    out = np.empty((B, 3, H, W), np.float32)
    for k in range(N_CORES):
        b, hh = k // 2, k % 2
        # scratch [6*NP, NMAC, F]: row 6p+3h+j, macro g, x
        #   -> pixel g*FM + (2p+h)*F + x of channel j
        G, FM, NMAC, NQ, NP = _geom()
        sc = results[k]["out"].reshape(NP, 2, 3, NMAC, F)
        core = sc.transpose(2, 3, 0, 1, 4).reshape(3, PIX)
        out[b, :, hh * 128:(hh + 1) * 128, :] = core.reshape(3, 128, W)
    return out


def kernel(input: np.ndarray, weight: np.ndarray, coor: np.ndarray) -> np.ndarray:
    nc = _get_nc(1)
    in_maps = make_in_maps(np.asarray(weight), np.asarray(coor))
    res = run_bass_kernel_spmd(nc, in_maps, core_ids=list(range(N_CORES)))
    return assemble_out(res.results)
